# revision 4
# baseline (speedup 1.0000x reference)
"""Trainium2 Bass kernel for nn_NeuronBank (moe_routing).

Problem: 4 neuron banks W_Q/W_K/W_V [N=2048, D=256, R=32] and W_O [N, R, D];
indices [B=2, S=512, K=4] select rows; outputs are per-token gathered banks
plus an orthogonality loss sum_banks mean_n ||W_n^T W_n - I_R||^2 / 4.

Strategy (8 NeuronCores, SPMD):
  - Token-parallel gather: each core handles 512 of the 4096 (b,s,k) slots.
    dma_gather (SWDGE indexed gather, 128 rows x 32KB per instruction)
    HBM -> SBUF, then HWDGE store SBUF -> HBM output slab.
  - Expert-parallel loss: each core computes Gram matrices for its 256-row
    shard of every bank. Rows are DMA'd contiguously ([128 rows, 8192]),
    PE-transposed (128x128 slices at fixed r) into [d, (neuron, r)] layout,
    then 4-neuron-packed W^T W matmuls accumulate over the two 128-deep d
    chunks in PSUM. (x - I)^2 * blockmask is reduced on DVE via
    tensor_tensor_reduce into a [128, 1] accumulator; the host sums the 8
    per-core partial vectors and normalizes.

The full (unsharded) inputs come in; sharding/replication and the final
concatenation/reduction happen on the host.
"""

import numpy as np

# Problem constants (hardcoded per the harness contract).
N, D, R = 2048, 256, 32
B, S, K = 2, 512, 4
E = D * R  # 8192 elements per bank row
NCORES = 8
TOK = B * S * K            # 4096 gathered rows per bank
TPC = TOK // NCORES        # 512 tokens per core
RPC = N // NCORES          # 256 loss rows (neurons) per core

_CACHE = {}


def build_program(n_bank=N, tok_pc=TPC, rows_pc=RPC):
    """Build and compile the SPMD Bass program. Returns (nc, meta)."""
    import concourse.bass as bass  # noqa: F401
    import concourse.tile as tile
    from concourse import bacc, mybir

    F32 = mybir.dt.float32
    I16 = mybir.dt.int16
    assert tok_pc % 128 == 0 and rows_pc % 128 == 0
    n_chunks = tok_pc // 128
    n_rowtiles = rows_pc // 128

    nc = bacc.Bacc("TRN2", target_bir_lowering=False, debug=False,
                   num_devices=NCORES)

    banks = [nc.dram_tensor(f"W{i}", [n_bank, E], F32, kind="ExternalInput")
             for i in range(4)]
    shards = [nc.dram_tensor(f"S{i}", [rows_pc, E], F32, kind="ExternalInput")
              for i in range(4)]
    idx_t = nc.dram_tensor("IDX", [128, tok_pc // 16], I16, kind="ExternalInput")
    ident_t = nc.dram_tensor("IDENT", [128, 128], F32, kind="ExternalInput")
    itile_t = nc.dram_tensor("ITILE", [128, 512], F32, kind="ExternalInput")
    mask_t = nc.dram_tensor("MASK", [128, 512], F32, kind="ExternalInput")
    outs = [nc.dram_tensor(f"O{i}", [tok_pc, E], F32, kind="ExternalOutput")
            for i in range(4)]
    loss_t = nc.dram_tensor("LOSS", [128, 1], F32, kind="ExternalOutput")

    sub = mybir.AluOpType.subtract
    mult = mybir.AluOpType.mult
    add = mybir.AluOpType.add

    with tile.TileContext(nc) as tc:
        with (
            tc.tile_pool(name="const", bufs=1) as const,
            tc.tile_pool(name="gather", bufs=2) as gpool,
            tc.tile_pool(name="rows", bufs=2) as rowpool,
            tc.tile_pool(name="L", bufs=1) as lpool,
            tc.tile_pool(name="scratch", bufs=2) as spool,
            tc.tile_pool(name="psT", bufs=2, space="PSUM") as psT,
            tc.tile_pool(name="psG", bufs=2, space="PSUM") as psG,
        ):
            ident = const.tile([128, 128], F32, tag="ident")
            nc.scalar.dma_start(ident[:], ident_t[:])
            itile = const.tile([128, 512], F32, tag="itile")
            nc.scalar.dma_start(itile[:], itile_t[:])
            mask = const.tile([128, 512], F32, tag="mask")
            nc.scalar.dma_start(mask[:], mask_t[:])
            idx = const.tile([128, tok_pc // 16], I16, tag="idx")
            nc.scalar.dma_start(idx[:], idx_t[:])
            accs = [const.tile([128, 1], F32, tag=f"acc{i}", name=f"acc{i}")
                    for i in range(2)]
            nc.vector.memset(accs[0][:], 0.0)

            round_i = 0
            for b in range(4):
                # ---- gather this bank's tokens ----
                for k in range(n_chunks):
                    g = gpool.tile([128, 1, E], F32, tag="g")
                    nc.gpsimd.dma_gather(
                        g[:], banks[b][:], idx[:, k * 8:(k + 1) * 8],
                        num_idxs=128, num_idxs_reg=128, elem_size=E,
                    )
                    nc.sync.dma_start(outs[b][k * 128:(k + 1) * 128, :],
                                      g[:, 0, :])

                # ---- loss over this bank's shard ----
                is_o = (b == 3)
                for t in range(n_rowtiles):
                    rt = rowpool.tile([128, E], F32, tag="rt")
                    nc.scalar.dma_start(rt[:],
                                        shards[b][t * 128:(t + 1) * 128, :])
                    if is_o:
                        rview = rt[:].rearrange("p (r d) -> p r d", d=D)
                    else:
                        rview = rt[:].rearrange("p (d r) -> p d r", r=R)
                    ls = [lpool.tile([128, 128 * R], F32, tag=f"L{c}",
                                     name=f"L{c}_{b}_{t}")
                          for c in range(2)]
                    for c in range(2):
                        for r4 in range(R // 4):
                            pT = psT.tile([128, 512], F32, tag="pT")
                            for rr in range(4):
                                r_abs = r4 * 4 + rr
                                if is_o:
                                    src = rview[:, r_abs, c * 128:(c + 1) * 128]
                                else:
                                    src = rview[:, c * 128:(c + 1) * 128, r_abs]
                                nc.tensor.transpose(
                                    pT[:, rr * 128:(rr + 1) * 128], src,
                                    ident[:])
                            dst = (ls[c][:]
                                   .rearrange("p (n r) -> p n r", r=R)
                                   [:, :, r4 * 4:(r4 + 1) * 4]
                                   .transpose([0, 2, 1]))
                            srcap = pT[:].rearrange("p (rr n) -> p rr n", n=128)
                            nc.vector.tensor_copy(dst, srcap)
                    for rnd in range(8):
                        pG = psG.tile([128, 512], F32, tag="pG")
                        for g2 in range(4):
                            gi = rnd * 4 + g2
                            for c in range(2):
                                lslice = ls[c][:, gi * 128:(gi + 1) * 128]
                                nc.tensor.matmul(
                                    pG[:, g2 * 128:(g2 + 1) * 128],
                                    lslice, lslice,
                                    start=(c == 0), stop=(c == 1))
                        d = spool.tile([128, 512], F32, tag="d")
                        dm = spool.tile([128, 512], F32, tag="dm")
                        sq = spool.tile([128, 512], F32, tag="sq")
                        rt1 = spool.tile([128, 1], F32, tag="rt1")
                        nc.vector.tensor_tensor(d[:], pG[:], itile[:], op=sub)
                        nc.vector.tensor_tensor(dm[:], d[:], mask[:], op=mult)
                        nc.vector.tensor_tensor(sq[:], d[:], dm[:], op=mult)
                        nc.vector.tensor_reduce(
                            rt1[:], sq[:], axis=mybir.AxisListType.X, op=add)
                        nc.vector.tensor_tensor(
                            accs[(round_i + 1) % 2][:],
                            accs[round_i % 2][:], rt1[:], op=add)
                        round_i += 1

            final_acc = accs[round_i % 2]
            nc.sync.dma_start(loss_t[:], final_acc[:])

    nc.compile()
    meta = dict(n_bank=n_bank, tok_pc=tok_pc, rows_pc=rows_pc,
                n_rounds=round_i)
    return nc, meta


def make_idx_table(idx_core):
    """[tok_pc] row ids -> dma_gather layout [128, tok_pc//16] int16.

    Index j lives at (partition j%16, column j//16); the 16-partition block
    is replicated 8x across the 128 partitions.
    """
    t = np.asarray(idx_core).astype(np.int16)
    tbl = t.reshape(-1, 16).T          # [16, tok_pc//16]
    return np.ascontiguousarray(np.tile(tbl, (8, 1)))


def make_consts():
    ident = np.eye(128, dtype=np.float32)
    itile = np.ascontiguousarray(np.tile(np.eye(128, dtype=np.float32), (1, 4)))
    blk = np.kron(np.eye(4, dtype=np.float32), np.ones((32, 32), np.float32))
    mtile = np.ascontiguousarray(np.tile(blk, (1, 4)))
    return ident, itile, mtile


def make_in_maps(indices, W_Q, W_K, W_V, W_O, n_bank=N, tok_pc=TPC,
                 rows_pc=RPC, ncores=NCORES):
    idx_flat = np.asarray(indices).reshape(-1).astype(np.int64)
    ws = [np.ascontiguousarray(
              np.asarray(w, dtype=np.float32).reshape(n_bank, E))
          for w in (W_Q, W_K, W_V, W_O)]
    ident, itile, mtile = make_consts()
    in_maps = []
    for c in range(ncores):
        m = {}
        for i, w in enumerate(ws):
            m[f"W{i}"] = w
            m[f"S{i}"] = np.ascontiguousarray(
                w[c * rows_pc:(c + 1) * rows_pc])
        m["IDX"] = make_idx_table(idx_flat[c * tok_pc:(c + 1) * tok_pc])
        m["IDENT"] = ident
        m["ITILE"] = itile
        m["MASK"] = mtile
        in_maps.append(m)
    return in_maps


def assemble(results, tok_pc=TPC):
    """results: per-core dict name->np.ndarray. Returns the reference tuple."""
    fulls = []
    for i in range(4):
        fulls.append(np.concatenate([r[f"O{i}"] for r in results], axis=0))
    wq = fulls[0].reshape(B, S, K, D, R)
    wk = fulls[1].reshape(B, S, K, D, R)
    wv = fulls[2].reshape(B, S, K, D, R)
    wo = fulls[3].reshape(B, S, K, R, D)
    acc = np.float64(0.0)
    for r in results:
        acc += np.asarray(r["LOSS"], dtype=np.float64).sum()
    loss = np.float32(acc / (4.0 * N * R * R))
    return wq, wk, wv, wo, loss


def kernel(indices, W_Q, W_K, W_V, W_O):
    from concourse import bass_utils

    if "nc" not in _CACHE:
        _CACHE["nc"], _CACHE["meta"] = build_program()
    nc = _CACHE["nc"]
    in_maps = make_in_maps(indices, W_Q, W_K, W_V, W_O)
    res = bass_utils.run_bass_kernel_spmd(nc, in_maps,
                                          core_ids=list(range(NCORES)))
    return assemble(res.results)
